# revision 7
# baseline (speedup 1.0000x reference)
"""Trainium2 Bass kernel for the Cheirality loss layer.

Math (per batch b, pixel (y, x); g = grad_dirs, n = normal_flow):
    AV0 = V2*x - V0                    AV1 = V2*y - V1
    BW0 = O0*x*y - O1*(x^2+1) + O2*y   BW1 = O0*(y^2+1) - O1*x*y - O2*x
    rho = (g0*AV0 + g1*AV1) * (n0 + n1 - g0*BW0 - g1*BW1)
    out = mean(gelu(-rho))             (exact erf-based gelu)

E-factored form used on device (validated vs the reference to 2e-5 with
bf16 intermediates):
    u  = g0*x + g1*y
    E  = O0*y - O1*x
    F1 = O2*y - O1
    F2 = -O2*x + O0
    g.BW = E*u + g0*F1 + g1*F2
    dot1 = V2*u - V0*g0 - V1*g1
    rho  = dot1 * (n0 + n1 - g.BW)
    gelu(-rho) = (-rho/2) * (1 + erf(-rho/sqrt(2)))

Sharding: pure data parallel, 2 batches per core. Each core's tile layout is
[128, 4800]: partitions 0-63 hold batch 2m's 307200 pixels (64 rows x 4800
contiguous pixels), partitions 64-127 hold batch 2m+1. All pose-derived
scalars become per-partition [128,1] operands, so one pass covers both
batches. x/y pixel-coordinate grids are streamed as fp16 (exact for ints
< 2048). Final reduction: ACT/DVE accum_out -> [128, 4] partial sums per
core, summed on host in float64.
"""

import numpy as np

import concourse.bacc as bacc
import concourse.bass as bass
import concourse.tile as tile
from concourse import mybir
from concourse.bass_utils import run_bass_kernel_spmd

# Problem geometry (hardcoded per the task contract).
B, H, W = 16, 480, 640
NPIX = H * W            # 307200
NCORES = 8
BPC = B // NCORES       # 2 batches per core
PHALF = 64              # partitions per batch
FTOT = NPIX // PHALF    # 4800 free elems per partition
FC = 1200               # chunk size along free dim
NCHUNK = FTOT // FC     # 4

F32 = mybir.dt.float32
F16 = mybir.dt.float16
BF16 = mybir.dt.bfloat16
ALU = mybir.AluOpType
AF = mybir.ActivationFunctionType
SQRT2 = float(np.sqrt(2.0))


def _build_kernel(tc, gd, nf, xg, yg, scal, out):
    nc = tc.nc
    gd_t = gd.ap().rearrange("b c (p f) -> b c p f", p=PHALF)
    nf_t = nf.ap().rearrange("b c (p f) -> b c p f", p=PHALF)
    xg_a = xg.ap()
    yg_a = yg.ap()

    with (
        tc.tile_pool(name="singles", bufs=1) as singles,
        tc.tile_pool(name="ins", bufs=3) as ins,
        tc.tile_pool(name="grids", bufs=2) as grids,
        tc.tile_pool(name="mids", bufs=2) as mids,
    ):
        sc = singles.tile([128, 8], F32, name="sc")
        nc.sync.dma_start(out=sc, in_=scal.ap())
        W0 = sc[:, 0:1]     # Omega0
        W1n = sc[:, 1:2]    # -Omega1
        W2 = sc[:, 2:3]     # Omega2
        W2n = sc[:, 3:4]    # -Omega2
        V0n = sc[:, 4:5]    # -V0
        V1n = sc[:, 5:6]    # -V1
        V2 = sc[:, 6:7]     # V2

        acc = singles.tile([128, NCHUNK], F32, name="acc")
        for ci in range(NCHUNK):
            sl = slice(ci * FC, (ci + 1) * FC)
            g0 = ins.tile([128, FC], F32, tag="g0", name=f"g0_{ci}")
            g1 = ins.tile([128, FC], F32, tag="g1", name=f"g1_{ci}")
            n0 = ins.tile([128, FC], F32, tag="n0", name=f"n0_{ci}")
            n1 = ins.tile([128, FC], F32, tag="n1", name=f"n1_{ci}")
            xt = grids.tile([128, FC], F16, tag="x", name=f"x_{ci}")
            yt = grids.tile([128, FC], F16, tag="y", name=f"y_{ci}")
            for h in range(BPC):
                ps = slice(h * PHALF, (h + 1) * PHALF)
                nc.sync.dma_start(out=xt[ps], in_=xg_a[:, sl])
                nc.sync.dma_start(out=yt[ps], in_=yg_a[:, sl])
                nc.sync.dma_start(out=g0[ps], in_=gd_t[h, 0, :, sl])
                nc.sync.dma_start(out=g1[ps], in_=gd_t[h, 1, :, sl])
                nc.sync.dma_start(out=n0[ps], in_=nf_t[h, 0, :, sl])
                nc.sync.dma_start(out=n1[ps], in_=nf_t[h, 1, :, sl])

            def mtile(tag, dt=BF16):
                return mids.tile([128, FC], dt, tag=tag, name=f"{tag}_{ci}")

            # bf16 copies of grad dirs (ACT; GPSIMD CAST measured 4.5us)
            g0b = mtile("g0b")
            nc.scalar.activation(out=g0b, in_=g0, func=AF.Copy)
            g1b = mtile("g1b")
            nc.scalar.activation(out=g1b, in_=g1, func=AF.Copy)

            # E = O0*y - O1*x  (e1 on ACT, e2 on GPS-ts, add on DVE)
            e1 = mtile("e1")
            nc.scalar.activation(out=e1, in_=yt, func=AF.Copy, bias=0.0, scale=W0)
            e2 = mtile("e2")
            nc.gpsimd.tensor_scalar_mul(out=e2, in0=xt, scalar1=W1n)
            E = e1  # in-place: E = e1 + e2
            nc.vector.tensor_add(out=E, in0=e1, in1=e2)
            # F1 = O2*y - O1 (ACT); F2 = -O2*x + O0 (DVE dual tensor_scalar)
            F1 = mtile("F1")
            nc.scalar.activation(out=F1, in_=yt, func=AF.Identity, bias=W1n, scale=W2)
            F2 = mtile("F2")
            nc.vector.tensor_scalar(
                out=F2, in0=xt, scalar1=W2n, scalar2=W0, op0=ALU.mult, op1=ALU.add
            )

            # u = g0*x + g1*y
            u1 = mtile("u1")
            nc.vector.tensor_mul(out=u1, in0=g0b, in1=xt)
            u2 = mtile("u2")
            nc.gpsimd.tensor_mul(out=u2, in0=g1b, in1=yt)
            u = u1  # in-place
            nc.vector.tensor_add(out=u, in0=u1, in1=u2)

            # g.BW = E*u + g0*F1 + g1*F2
            q = mtile("q")
            nc.vector.tensor_mul(out=q, in0=E, in1=u)
            p1 = mtile("p1")
            nc.vector.tensor_mul(out=p1, in0=g0b, in1=F1)
            t = q  # in-place: t = q + p1
            nc.vector.tensor_add(out=t, in0=q, in1=p1)
            p2 = mtile("p2")
            nc.gpsimd.tensor_mul(out=p2, in0=g1b, in1=F2)
            t2 = t  # in-place: t2 = t + p2
            nc.vector.tensor_add(out=t2, in0=t, in1=p2)

            # r2 = n0 + n1 - g.BW
            s = mtile("s")
            nc.gpsimd.tensor_add(out=s, in0=n0, in1=n1)
            r2 = s  # in-place
            nc.vector.tensor_sub(out=r2, in0=s, in1=t2)

            # dot1 = V2*u - V0*g0 - V1*g1 (fused stt chain for accuracy)
            m0 = mtile("m0")
            nc.scalar.activation(out=m0, in_=g0, func=AF.Copy, bias=0.0, scale=V0n)
            m = m0  # in-place
            nc.vector.scalar_tensor_tensor(
                out=m, in0=g1b, scalar=V1n, in1=m0, op0=ALU.mult, op1=ALU.add
            )
            dot1 = mtile("dot1")
            nc.vector.scalar_tensor_tensor(
                out=dot1, in0=u, scalar=V2, in1=m, op0=ALU.mult, op1=ALU.add
            )

            # rho = dot1 * r2 ; out += gelu(-rho) via native ACT gelu
            rho = dot1  # in-place
            nc.vector.tensor_mul(out=rho, in0=dot1, in1=r2)
            gl = mtile("gl")
            nc.scalar.activation(
                out=gl, in_=rho, func=AF.Gelu, bias=0.0, scale=-1.0,
                accum_out=acc[:, ci : ci + 1],
            )

        nc.sync.dma_start(out=out.ap(), in_=acc)


def build_bass():
    nc = bacc.Bacc("TRN2", target_bir_lowering=False, debug=False)
    gd = nc.dram_tensor("gd", [BPC, 2, NPIX], F32, kind="ExternalInput")
    nf = nc.dram_tensor("nf", [BPC, 2, NPIX], F32, kind="ExternalInput")
    xg = nc.dram_tensor("xg", [PHALF, FTOT], F16, kind="ExternalInput")
    yg = nc.dram_tensor("yg", [PHALF, FTOT], F16, kind="ExternalInput")
    scal = nc.dram_tensor("scal", [128, 8], F32, kind="ExternalInput")
    out = nc.dram_tensor("acc_out", [128, NCHUNK], F32, kind="ExternalOutput")
    with tile.TileContext(nc) as tc:
        _build_kernel(tc, gd, nf, xg, yg, scal, out)
    nc.compile()
    return nc


def make_in_maps(pose, grad_dirs, normal_flow):
    pose = np.asarray(pose, np.float32)
    gd = np.ascontiguousarray(np.asarray(grad_dirs, np.float32)).reshape(B, 2, NPIX)
    nf = np.ascontiguousarray(np.asarray(normal_flow, np.float32)).reshape(B, 2, NPIX)

    flat = np.arange(NPIX, dtype=np.int64).reshape(PHALF, FTOT)
    xg = (flat % W).astype(np.float16)
    yg = (flat // W).astype(np.float16)

    in_maps = []
    for core in range(NCORES):
        b0 = core * BPC
        sc = np.zeros((128, 8), np.float32)
        for h in range(BPC):
            V = pose[b0 + h, :3]
            O = pose[b0 + h, 3:]
            rows = slice(h * PHALF, (h + 1) * PHALF)
            sc[rows, 0] = O[0]
            sc[rows, 1] = -O[1]
            sc[rows, 2] = O[2]
            sc[rows, 3] = -O[2]
            sc[rows, 4] = -V[0]
            sc[rows, 5] = -V[1]
            sc[rows, 6] = V[2]
        in_maps.append(
            {
                "gd": np.ascontiguousarray(gd[b0 : b0 + BPC]),
                "nf": np.ascontiguousarray(nf[b0 : b0 + BPC]),
                "xg": xg,
                "yg": yg,
                "scal": sc,
            }
        )
    return in_maps


_NC_CACHE = None


def _get_nc():
    global _NC_CACHE
    if _NC_CACHE is None:
        _NC_CACHE = build_bass()
    return _NC_CACHE


def kernel(pose, grad_dirs, normal_flow):
    nc = _get_nc()
    in_maps = make_in_maps(pose, grad_dirs, normal_flow)
    res = run_bass_kernel_spmd(nc, in_maps, core_ids=list(range(NCORES)))
    total = 0.0
    for r in res.results:
        total += r["acc_out"].astype(np.float64).sum()
    return np.float32(total / (B * H * W))


# revision 14
# speedup vs baseline: 1.5118x; 1.5118x over previous
"""Trainium2 Bass kernel for the Cheirality loss layer.

Math (per batch b, pixel (y, x); g = grad_dirs, n = normal_flow):
    AV0 = V2*x - V0                    AV1 = V2*y - V1
    BW0 = O0*x*y - O1*(x^2+1) + O2*y   BW1 = O0*(y^2+1) - O1*x*y - O2*x
    rho = (g0*AV0 + g1*AV1) * (n0 + n1 - g0*BW0 - g1*BW1)
    out = mean(gelu(-rho))             (exact erf-based gelu)

E-factored form used on device (validated vs the reference to 2e-5 with
bf16 intermediates):
    u  = g0*x + g1*y
    E  = O0*y - O1*x
    F1 = O2*y - O1
    F2 = -O2*x + O0
    g.BW = E*u + g0*F1 + g1*F2
    dot1 = V2*u - V0*g0 - V1*g1
    rho  = dot1 * (n0 + n1 - g.BW)
    gelu(-rho) = (-rho/2) * (1 + erf(-rho/sqrt(2)))

Sharding: pure data parallel, 2 batches per core. Each core's tile layout is
[128, 4800]: partitions 0-63 hold batch 2m's 307200 pixels (64 rows x 4800
contiguous pixels), partitions 64-127 hold batch 2m+1. All pose-derived
scalars become per-partition [128,1] operands, so one pass covers both
batches. x/y pixel-coordinate grids are streamed as fp16 (exact for ints
< 2048). Final reduction: ACT/DVE accum_out -> [128, 4] partial sums per
core, summed on host in float64.
"""

import numpy as np

import concourse.bacc as bacc
import concourse.bass as bass
import concourse.tile as tile
from concourse import mybir
from concourse.bass_utils import run_bass_kernel_spmd

# Problem geometry (hardcoded per the task contract).
B, H, W = 16, 480, 640
NPIX = H * W            # 307200
NCORES = 8
BPC = B // NCORES       # 2 batches per core
PHALF = 64              # partitions per batch
FTOT = NPIX // PHALF    # 4800 free elems per partition
FC = 1200               # chunk size along free dim
NCHUNK = FTOT // FC     # 4

F32 = mybir.dt.float32
F16 = mybir.dt.float16
BF16 = mybir.dt.bfloat16
ALU = mybir.AluOpType
AF = mybir.ActivationFunctionType
SQRT2 = float(np.sqrt(2.0))


def _build_kernel(tc, gd, nf, xyg, scal, out):
    nc = tc.nc
    # [b, c, p, f] with p the partition block per batch
    gd_t = gd.ap().rearrange("b c (p f) -> b c p f", p=PHALF)
    nf_t = nf.ap().rearrange("b c (p f) -> b c p f", p=PHALF)
    # [p, c, f] view so one 3-dim DMA covers both channels of one batch
    xy_t = xyg.ap().rearrange("c p f -> p c f")  # [64, 2, FTOT]

    with (
        tc.tile_pool(name="singles", bufs=1) as singles,
        tc.tile_pool(name="ins", bufs=3) as ins,
        tc.tile_pool(name="grids", bufs=2) as grids,
        tc.tile_pool(name="mids", bufs=2) as mids,
    ):
        sc = singles.tile([128, 8], F32, name="sc")
        nc.sync.dma_start(out=sc, in_=scal.ap())
        W0 = sc[:, 0:1]     # Omega0
        W1n = sc[:, 1:2]    # -Omega1
        W2 = sc[:, 2:3]     # Omega2
        W2n = sc[:, 3:4]    # -Omega2
        V0n = sc[:, 4:5]    # -V0
        V1n = sc[:, 5:6]    # -V1
        V2 = sc[:, 6:7]     # V2

        acc = singles.tile([128, NCHUNK], F32, name="acc")
        for ci in range(NCHUNK):
            sl = slice(ci * FC, (ci + 1) * FC)
            # fused 128-partition DMAs: [b, c, p, f] -> [(b p), c, f]
            gdt = ins.tile([128, 2, FC], F32, tag="gdt", name=f"gdt_{ci}")
            nft = ins.tile([128, 2, FC], F32, tag="nft", name=f"nft_{ci}")
            xyt = grids.tile([128, 2, FC], F16, tag="xy", name=f"xy_{ci}")
            for h in range(BPC):
                ps = slice(h * PHALF, (h + 1) * PHALF)
                nc.sync.dma_start(out=xyt[ps], in_=xy_t[:, :, sl])
                nc.sync.dma_start(
                    out=gdt[ps], in_=gd_t[h].rearrange("c p f -> p c f")[:, :, sl]
                )
                nc.sync.dma_start(
                    out=nft[ps], in_=nf_t[h].rearrange("c p f -> p c f")[:, :, sl]
                )
            g0 = gdt[:, 0]
            g1 = gdt[:, 1]
            n0 = nft[:, 0]
            n1 = nft[:, 1]
            xt = xyt[:, 0]
            yt = xyt[:, 1]

            def mtile(tag, dt=BF16):
                return mids.tile([128, FC], dt, tag=tag, name=f"{tag}_{ci}")

            # bf16 copies of grad dirs (ACT; GPSIMD CAST measured 4.5us)
            g0b = mtile("g0b")
            nc.scalar.activation(out=g0b, in_=g0, func=AF.Copy)
            g1b = mtile("g1b")
            nc.scalar.activation(out=g1b, in_=g1, func=AF.Copy)

            # E = O0*y - O1*x   (ACT for the y part, fused stt for the rest)
            e1 = mtile("e1")
            nc.scalar.activation(out=e1, in_=yt, func=AF.Copy, bias=0.0, scale=W0)
            E = e1  # in-place
            nc.vector.scalar_tensor_tensor(
                out=E, in0=xt, scalar=W1n, in1=e1, op0=ALU.mult, op1=ALU.add
            )
            # F1 = O2*y - O1 ; F2 = -O2*x + O0  (ACT affine)
            F1 = mtile("F1")
            nc.scalar.activation(out=F1, in_=yt, func=AF.Identity, bias=W1n, scale=W2)
            F2 = mtile("F2")
            nc.scalar.activation(out=F2, in_=xt, func=AF.Identity, bias=W0, scale=W2n)

            # u = g0*x + g1*y
            u1 = mtile("u1")
            nc.vector.tensor_mul(out=u1, in0=g0b, in1=xt)
            u2 = mtile("u2")
            nc.gpsimd.tensor_mul(out=u2, in0=g1b, in1=yt)
            u = u1  # in-place
            nc.vector.tensor_add(out=u, in0=u1, in1=u2)

            # g.BW = E*u + g0*F1 + g1*F2
            q = mtile("q")
            nc.vector.tensor_mul(out=q, in0=E, in1=u)
            p1 = mtile("p1")
            nc.vector.tensor_mul(out=p1, in0=g0b, in1=F1)
            t = q  # in-place: t = q + p1
            nc.vector.tensor_add(out=t, in0=q, in1=p1)
            p2 = mtile("p2")
            nc.gpsimd.tensor_mul(out=p2, in0=g1b, in1=F2)
            t2 = t  # in-place: t2 = t + p2
            nc.vector.tensor_add(out=t2, in0=t, in1=p2)

            # r2 = n0 + n1 - g.BW
            s = mtile("s")
            nc.gpsimd.tensor_add(out=s, in0=n0, in1=n1)
            r2 = s  # in-place
            nc.vector.tensor_sub(out=r2, in0=s, in1=t2)

            # dot1 = V2*u - V0*g0 - V1*g1 (fused stt chain for accuracy)
            m0 = mtile("m0")
            nc.scalar.activation(out=m0, in_=g0, func=AF.Copy, bias=0.0, scale=V0n)
            m = m0  # in-place
            nc.vector.scalar_tensor_tensor(
                out=m, in0=g1b, scalar=V1n, in1=m0, op0=ALU.mult, op1=ALU.add
            )
            dot1 = mtile("dot1")
            nc.vector.scalar_tensor_tensor(
                out=dot1, in0=u, scalar=V2, in1=m, op0=ALU.mult, op1=ALU.add
            )

            # rho = dot1 * r2 ; out += gelu(-rho) via native ACT gelu
            rho = dot1  # in-place
            nc.vector.tensor_mul(out=rho, in0=dot1, in1=r2)
            gl = mtile("gl")
            nc.scalar.activation(
                out=gl, in_=rho, func=AF.Gelu, bias=0.0, scale=-1.0,
                accum_out=acc[:, ci : ci + 1],
            )

        nc.sync.dma_start(out=out.ap(), in_=acc)


def build_bass():
    nc = bacc.Bacc("TRN2", target_bir_lowering=False, debug=False)
    gd = nc.dram_tensor("gd", [BPC, 2, NPIX], F32, kind="ExternalInput")
    nf = nc.dram_tensor("nf", [BPC, 2, NPIX], F32, kind="ExternalInput")
    xyg = nc.dram_tensor("xyg", [2, PHALF, FTOT], F16, kind="ExternalInput")
    scal = nc.dram_tensor("scal", [128, 8], F32, kind="ExternalInput")
    out = nc.dram_tensor("acc_out", [128, NCHUNK], F32, kind="ExternalOutput")
    with tile.TileContext(nc) as tc:
        _build_kernel(tc, gd, nf, xyg, scal, out)
    nc.compile()
    return nc


def make_in_maps(pose, grad_dirs, normal_flow):
    pose = np.asarray(pose, np.float32)
    gd = np.ascontiguousarray(np.asarray(grad_dirs, np.float32)).reshape(B, 2, NPIX)
    nf = np.ascontiguousarray(np.asarray(normal_flow, np.float32)).reshape(B, 2, NPIX)

    flat = np.arange(NPIX, dtype=np.int64).reshape(PHALF, FTOT)
    xyg = np.stack([(flat % W), (flat // W)]).astype(np.float16)

    in_maps = []
    for core in range(NCORES):
        b0 = core * BPC
        sc = np.zeros((128, 8), np.float32)
        for h in range(BPC):
            V = pose[b0 + h, :3]
            O = pose[b0 + h, 3:]
            rows = slice(h * PHALF, (h + 1) * PHALF)
            sc[rows, 0] = O[0]
            sc[rows, 1] = -O[1]
            sc[rows, 2] = O[2]
            sc[rows, 3] = -O[2]
            sc[rows, 4] = -V[0]
            sc[rows, 5] = -V[1]
            sc[rows, 6] = V[2]
        in_maps.append(
            {
                "gd": np.ascontiguousarray(gd[b0 : b0 + BPC]),
                "nf": np.ascontiguousarray(nf[b0 : b0 + BPC]),
                "xyg": xyg,
                "scal": sc,
            }
        )
    return in_maps


_NC_CACHE = None


def _get_nc():
    global _NC_CACHE
    if _NC_CACHE is None:
        _NC_CACHE = build_bass()
    return _NC_CACHE


def kernel(pose, grad_dirs, normal_flow):
    nc = _get_nc()
    in_maps = make_in_maps(pose, grad_dirs, normal_flow)
    res = run_bass_kernel_spmd(nc, in_maps, core_ids=list(range(NCORES)))
    total = 0.0
    for r in res.results:
        total += r["acc_out"].astype(np.float64).sum()
    return np.float32(total / (B * H * W))


# revision 19
# speedup vs baseline: 1.9506x; 1.2903x over previous
"""Trainium2 Bass kernel for the Cheirality loss layer.

Math (per batch b, pixel (y, x); g = grad_dirs, n = normal_flow):
    AV0 = V2*x - V0                    AV1 = V2*y - V1
    BW0 = O0*x*y - O1*(x^2+1) + O2*y   BW1 = O0*(y^2+1) - O1*x*y - O2*x
    rho = (g0*AV0 + g1*AV1) * (n0 + n1 - g0*BW0 - g1*BW1)
    out = mean(gelu(-rho))             (exact erf-based gelu)

E-factored form used on device (validated vs the reference to 2e-5 with
bf16 intermediates):
    u  = g0*x + g1*y
    E  = O0*y - O1*x
    F1 = O2*y - O1
    F2 = -O2*x + O0
    g.BW = E*u + g0*F1 + g1*F2
    dot1 = V2*u - V0*g0 - V1*g1
    rho  = dot1 * (n0 + n1 - g.BW)
    gelu(-rho) = (-rho/2) * (1 + erf(-rho/sqrt(2)))

Sharding: pure data parallel, 2 batches per core. Each core's tile layout is
[128, 4800]: partitions 0-63 hold batch 2m's 307200 pixels (64 rows x 4800
contiguous pixels), partitions 64-127 hold batch 2m+1. All pose-derived
scalars become per-partition [128,1] operands, so one pass covers both
batches. x/y pixel-coordinate grids are streamed as fp16 (exact for ints
< 2048). Final reduction: ACT/DVE accum_out -> [128, 4] partial sums per
core, summed on host in float64.
"""

import numpy as np

import concourse.bacc as bacc
import concourse.bass as bass
import concourse.tile as tile
from concourse import mybir
from concourse.bass_utils import run_bass_kernel_spmd

# Problem geometry (hardcoded per the task contract).
B, H, W = 16, 480, 640
NPIX = H * W            # 307200
NCORES = 8
BPC = B // NCORES       # 2 batches per core
PHALF = 64              # partitions per batch
FTOT = NPIX // PHALF    # 4800 free elems per partition
FC = 1200               # chunk size along free dim
NCHUNK = FTOT // FC     # 4

F32 = mybir.dt.float32
F16 = mybir.dt.float16
BF16 = mybir.dt.bfloat16
ALU = mybir.AluOpType
AF = mybir.ActivationFunctionType
SQRT2 = float(np.sqrt(2.0))


def _build_kernel(tc, gd, nf, xyg, scal, out):
    # Host pre-interleaves everything to [128, 2(c), FTOT] so each chunk is a
    # single 128-partition DMA per tensor (all 16 SDMA ports engaged).
    nc = tc.nc
    gd_t = gd.ap()
    nf_t = nf.ap()
    xy_t = xyg.ap()

    with (
        tc.tile_pool(name="singles", bufs=1) as singles,
        tc.tile_pool(name="ins", bufs=3) as ins,
        tc.tile_pool(name="grids", bufs=2) as grids,
        tc.tile_pool(name="mids", bufs=2) as mids,
    ):
        sc = singles.tile([128, 8], F32, name="sc")
        nc.sync.dma_start(out=sc, in_=scal.ap())
        W0 = sc[:, 0:1]     # Omega0
        W1n = sc[:, 1:2]    # -Omega1
        W2 = sc[:, 2:3]     # Omega2
        W2n = sc[:, 3:4]    # -Omega2
        V0n = sc[:, 4:5]    # -V0
        V1n = sc[:, 5:6]    # -V1
        V2 = sc[:, 6:7]     # V2

        acc = singles.tile([128, NCHUNK], F32, name="acc")
        for ci in range(NCHUNK):
            sl = slice(ci * FC, (ci + 1) * FC)
            # fused 128-partition DMAs: [b, c, p, f] -> [(b p), c, f]
            gdt = ins.tile([128, 2, FC], F32, tag="gdt", name=f"gdt_{ci}")
            nft = ins.tile([128, 2, FC], F32, tag="nft", name=f"nft_{ci}")
            xyt = grids.tile([128, 2, FC], F16, tag="xy", name=f"xy_{ci}")
            nc.sync.dma_start(out=xyt, in_=xy_t[:, :, sl])
            nc.sync.dma_start(out=gdt, in_=gd_t[:, :, sl])
            nc.sync.dma_start(out=nft, in_=nf_t[:, :, sl])
            g0 = gdt[:, 0]
            g1 = gdt[:, 1]
            n0 = nft[:, 0]
            n1 = nft[:, 1]
            xt = xyt[:, 0]
            yt = xyt[:, 1]

            def mtile(tag, dt=BF16):
                return mids.tile([128, FC], dt, tag=tag, name=f"{tag}_{ci}")

            # bf16 copies of grad dirs (ACT; GPSIMD CAST measured 4.5us)
            g0b = mtile("g0b")
            nc.scalar.activation(out=g0b, in_=g0, func=AF.Copy)
            g1b = mtile("g1b")
            nc.scalar.activation(out=g1b, in_=g1, func=AF.Copy)

            # E = O0*y - O1*x   (ACT for the y part, fused stt for the rest)
            e1 = mtile("e1")
            nc.scalar.activation(out=e1, in_=yt, func=AF.Copy, bias=0.0, scale=W0)
            E = e1  # in-place
            nc.vector.scalar_tensor_tensor(
                out=E, in0=xt, scalar=W1n, in1=e1, op0=ALU.mult, op1=ALU.add
            )
            # F1 = O2*y - O1 ; F2 = -O2*x + O0  (ACT affine)
            F1 = mtile("F1")
            nc.scalar.activation(out=F1, in_=yt, func=AF.Identity, bias=W1n, scale=W2)
            F2 = mtile("F2")
            nc.scalar.activation(out=F2, in_=xt, func=AF.Identity, bias=W0, scale=W2n)

            # u = g0*x + g1*y
            u1 = mtile("u1")
            nc.vector.tensor_mul(out=u1, in0=g0b, in1=xt)
            u2 = mtile("u2")
            nc.gpsimd.tensor_mul(out=u2, in0=g1b, in1=yt)
            u = u1  # in-place
            nc.vector.tensor_add(out=u, in0=u1, in1=u2)

            # g.BW = E*u + g0*F1 + g1*F2
            q = mtile("q")
            nc.vector.tensor_mul(out=q, in0=E, in1=u)
            p1 = mtile("p1")
            nc.vector.tensor_mul(out=p1, in0=g0b, in1=F1)
            t = q  # in-place: t = q + p1
            nc.vector.tensor_add(out=t, in0=q, in1=p1)
            p2 = mtile("p2")
            nc.gpsimd.tensor_mul(out=p2, in0=g1b, in1=F2)
            t2 = t  # in-place: t2 = t + p2
            nc.vector.tensor_add(out=t2, in0=t, in1=p2)

            # r2 = n0 + n1 - g.BW
            s = mtile("s")
            nc.gpsimd.tensor_add(out=s, in0=n0, in1=n1)
            r2 = s  # in-place
            nc.vector.tensor_sub(out=r2, in0=s, in1=t2)

            # dot1 = V2*u - V0*g0 - V1*g1 (fused stt chain for accuracy)
            m0 = mtile("m0")
            nc.scalar.activation(out=m0, in_=g0, func=AF.Copy, bias=0.0, scale=V0n)
            m = m0  # in-place
            nc.vector.scalar_tensor_tensor(
                out=m, in0=g1b, scalar=V1n, in1=m0, op0=ALU.mult, op1=ALU.add
            )
            dot1 = mtile("dot1")
            nc.vector.scalar_tensor_tensor(
                out=dot1, in0=u, scalar=V2, in1=m, op0=ALU.mult, op1=ALU.add
            )

            # rho = dot1 * r2 ; out += gelu(-rho) via native ACT gelu
            rho = dot1  # in-place
            nc.vector.tensor_mul(out=rho, in0=dot1, in1=r2)
            gl = mtile("gl")
            nc.scalar.activation(
                out=gl, in_=rho, func=AF.Gelu, bias=0.0, scale=-1.0,
                accum_out=acc[:, ci : ci + 1],
            )

        nc.sync.dma_start(out=out.ap(), in_=acc)


def build_bass():
    nc = bacc.Bacc("TRN2", target_bir_lowering=False, debug=False)
    gd = nc.dram_tensor("gd", [128, 2, FTOT], F32, kind="ExternalInput")
    nf = nc.dram_tensor("nf", [128, 2, FTOT], F32, kind="ExternalInput")
    xyg = nc.dram_tensor("xyg", [128, 2, FTOT], F16, kind="ExternalInput")
    scal = nc.dram_tensor("scal", [128, 8], F32, kind="ExternalInput")
    out = nc.dram_tensor("acc_out", [128, NCHUNK], F32, kind="ExternalOutput")
    with tile.TileContext(nc) as tc:
        _build_kernel(tc, gd, nf, xyg, scal, out)
    nc.compile()
    return nc


def make_in_maps(pose, grad_dirs, normal_flow):
    pose = np.asarray(pose, np.float32)
    gd = np.ascontiguousarray(np.asarray(grad_dirs, np.float32)).reshape(B, 2, NPIX)
    nf = np.ascontiguousarray(np.asarray(normal_flow, np.float32)).reshape(B, 2, NPIX)

    flat = np.arange(NPIX, dtype=np.int64).reshape(PHALF, FTOT)
    xy_half = np.stack([(flat % W), (flat // W)], axis=1)  # [64, 2, FTOT]
    xyg = np.ascontiguousarray(
        np.concatenate([xy_half, xy_half], axis=0).astype(np.float16)
    )  # [128, 2, FTOT]

    def interleave(a):
        # [BPC, 2, NPIX] -> [128, 2, FTOT]: batch b -> partitions [64b, 64b+64)
        return np.ascontiguousarray(
            a.reshape(BPC, 2, PHALF, FTOT).transpose(0, 2, 1, 3).reshape(128, 2, FTOT)
        )

    in_maps = []
    for core in range(NCORES):
        b0 = core * BPC
        sc = np.zeros((128, 8), np.float32)
        for h in range(BPC):
            V = pose[b0 + h, :3]
            O = pose[b0 + h, 3:]
            rows = slice(h * PHALF, (h + 1) * PHALF)
            sc[rows, 0] = O[0]
            sc[rows, 1] = -O[1]
            sc[rows, 2] = O[2]
            sc[rows, 3] = -O[2]
            sc[rows, 4] = -V[0]
            sc[rows, 5] = -V[1]
            sc[rows, 6] = V[2]
        in_maps.append(
            {
                "gd": interleave(gd[b0 : b0 + BPC]),
                "nf": interleave(nf[b0 : b0 + BPC]),
                "xyg": xyg,
                "scal": sc,
            }
        )
    return in_maps


_NC_CACHE = None


def _get_nc():
    global _NC_CACHE
    if _NC_CACHE is None:
        _NC_CACHE = build_bass()
    return _NC_CACHE


def kernel(pose, grad_dirs, normal_flow):
    nc = _get_nc()
    in_maps = make_in_maps(pose, grad_dirs, normal_flow)
    res = run_bass_kernel_spmd(nc, in_maps, core_ids=list(range(NCORES)))
    total = 0.0
    for r in res.results:
        total += r["acc_out"].astype(np.float64).sum()
    return np.float32(total / (B * H * W))
